# revision 113
# baseline (speedup 1.0000x reference)
"""Trainium2 Bass kernel for block-diagonal (per-graph) multi-head attention.

Full inputs in, full output out. Host side: graphs (contiguous segments of
the sorted node dim) are assigned whole to 8 NeuronCores (2 padded slots per
core, boustrophedon by size), weights replicated, x pre-transposed; outputs
are gathered back and the foldable biases (bv via softmax-rows-sum-to-1, bo)
are applied on the host.

Device program (SPMD, one compiled program, per-core data). Key structure
(v2 — overlap-oriented rewrite of the phase-sequential baseline; emission
order == per-engine execution order since engines execute their queues
in order, so the item pipeline is interleaved at emission time):
  - consolidated DMAs (HWDGE fixed cost is per instruction): xT in two
    slot-aligned column-halves on the ACT queue, wq/wk in output halves +
    wv/wo whole on the SP queue, small tensors on the Pool/SWDGE queue;
    wo and y in bf16 to halve bytes
  - the per-node 0/1 "vones" mask (excludes padded keys from both the AV
    numerator and the softmax denominator) is ONE row DMA + tiny Pool
    free-dim-broadcast copies into the packed V tiles' 65th column
  - front phase, all in one PSUM scope (qk 2 banks with per-slot-tile
    eviction ping-pong + scores0 mg0 + AV/V 2 + recip-broadcast 1):
    QK projection groups interleaved with slot0's scores/exp/AV/norm
    item pipeline; V projections injected mid-stream once wv lands
  - per (slot, head): scores^T -> ONE merged exp on ACT (scale=1/8) ->
    AV with the ones-column emitting the softmax denominator as row 64
  - normalization pipelined into each slot's pass: pair-merged
    reciprocals ([1, 2s] over head-pair-adjacent columns of o65all),
    PE-broadcast of 1/d across 64 partitions, per-item multiply into
    bf16 ots (odd heads into a staging half of the same tile, then one
    SBUF->SBUF DMA shift to partition base 64 on the Pool queue —
    compute engines cannot shift partitions)
  - back phase (scores1 + yproj 2 + recip-broadcast 2 banks): slot1's
    pipeline with slot0's per-chunk output projections + y DMAs
    injected; slot1 skips the shift for its last two head pairs — their
    yproj reads the odd staging directly via a 64-row contraction split
    against an extra copy of wo's odd rows (wo_odd)
  - engine balance: Q evictions + slot1-odd o65 copies + y evictions on
    ACT; K evictions + V copies + o65 copies + reciprocals + normalize
    muls on DVE; vones broadcasts + shifts on Pool
  - output projections per 128-node chunk, staged to bf16 and DMAd per
    chunk into a 128-aligned y layout
"""

import os
import sys

import ml_dtypes
import numpy as np

for _p in ("/opt/trn_rl_repo", os.path.expanduser("~/.axon_site/_ro/trn_rl_repo")):
    if os.path.isdir(_p) and _p not in sys.path:
        sys.path.insert(0, _p)

import concourse.bacc as bacc
import concourse.bass as bass
import concourse.mybir as mybir
import concourse.tile as tile
from concourse.bass_utils import run_bass_kernel_spmd

N_CORES = 8
HIDDEN = 512
NUM_HEADS = 8
HEAD_DIM = 64
KC = HIDDEN // 128  # contraction chunks of 128
F32 = mybir.dt.float32
F32R = mybir.dt.float32r
BF16 = mybir.dt.bfloat16
MAX_FREE = 512  # psum bank limit for fp32 free dim

_CACHE: dict = {}


def _ceil_div(a, b):
    return -(-a // b)


def _split_free(n, max_w=MAX_FREE):
    """Split even n into nearly equal EVEN pieces <= max_w (f32r matmuls
    need an even moving free dim)."""
    assert n % 2 == 0, n
    h = n // 2
    k = _ceil_div(n, max_w)
    base = h // k
    rem = h - base * k
    out = []
    off = 0
    for i in range(k):
        w = 2 * (base + (1 if i < rem else 0))
        out.append((off, w))
        off += w
    return out


def _build_program(slot_sizes):
    """Build + compile the SPMD Bass program for padded slot sizes."""
    G = len(slot_sizes)
    offs = [0]
    for s in slot_sizes:
        offs.append(offs[-1] + s)
    nc_tot = offs[-1]

    mchunks = {}
    for g in range(G):
        s = slot_sizes[g]
        mchunks[g] = [(mi * 128, min(128, s - mi * 128))
                      for mi in range(_ceil_div(s, 128))]
    mg = {g: len(mchunks[g]) for g in range(G)}
    # K columns are read in full-128 chunks by the scores matmuls; zero-pad
    pad_tail = max(0, _ceil_div(slot_sizes[-1], 128) * 128 - slot_sizes[-1])
    nck = nc_tot + pad_tail

    # 128-aligned per-slot offsets (for vones staging and the y layout)
    aoff = [0]
    for g in range(G):
        aoff.append(aoff[-1] + mg[g] * 128)
    tot_al = aoff[-1]
    totch = tot_al // 128

    # o65all column layout: pair-major so a head-pair's (even|odd) columns
    # are adjacent -> one reciprocal per pair
    gcol = {}
    oc = 0
    for g in range(G):
        gcol[g] = oc
        oc += 2 * NUM_HEADS // 2 * slot_sizes[g]
    o_total = oc

    def icol(g, h):
        return gcol[g] + (h // 2) * 2 * slot_sizes[g] + (h % 2) * slot_sizes[g]

    # Q/K eviction tiles aligned to slot boundaries: the first slot's
    # scores then depend only on the first tile's eviction
    if max(slot_sizes) <= MAX_FREE:
        n_tiles_all = [(offs[g], slot_sizes[g]) for g in range(G)]
    else:
        n_tiles_all = _split_free(nc_tot)

    nc = bacc.Bacc("TRN2", target_bir_lowering=False, debug=False,
                   num_devices=N_CORES)

    xT = nc.dram_tensor("xT", [HIDDEN, nc_tot], BF16, kind="ExternalInput")
    wq = nc.dram_tensor("wq", [HIDDEN, HIDDEN], BF16, kind="ExternalInput")
    wk = nc.dram_tensor("wk", [HIDDEN, HIDDEN], BF16, kind="ExternalInput")
    wv = nc.dram_tensor("wv", [HIDDEN, HIDDEN], BF16, kind="ExternalInput")
    wo = nc.dram_tensor("wo", [HIDDEN, HIDDEN], BF16, kind="ExternalInput")
    # odd-head rows of wo's dc2/dc3 chunks again, loaded at partitions
    # 0-63: lets the last head-pairs' yproj read the odd-head staging
    # (partitions 0-63) without waiting for the partition-shift DMA
    wo_odd = nc.dram_tensor("wo_odd", [128, HIDDEN], BF16,
                            kind="ExternalInput")
    bqk = nc.dram_tensor("bqk", [2 * KC, 128], F32, kind="ExternalInput")
    vones = nc.dram_tensor("vones", [tot_al], F32, kind="ExternalInput")
    y = nc.dram_tensor("y", [tot_al, HIDDEN], BF16, kind="ExternalOutput")

    with tile.TileContext(nc) as tc:
        with (
            tc.tile_pool(name="persist", bufs=1) as pp,
            tc.tile_pool(name="ework", bufs=8) as ep,
            tc.tile_pool(name="vwork", bufs=3) as vp,
        ):
            # ---- persistent tiles ------------------------------------
            xt_all = pp.tile([128, KC, nc_tot], BF16, tag="xt", name="xt")
            wq_all = pp.tile([128, KC, HIDDEN], BF16, tag="wq", name="wq_t")
            wk_all = pp.tile([128, KC, HIDDEN], BF16, tag="wk", name="wk_t")
            wv_all = pp.tile([128, KC, HIDDEN], BF16, tag="wv", name="wv_t")
            wo_all = pp.tile([128, KC, HIDDEN], BF16, tag="wo", name="wo_t")
            wo_odd_t = pp.tile([64, 2, HIDDEN], BF16, tag="wood",
                               name="wo_odd")
            bqk_t = pp.tile([128, 2 * KC], F32, tag="bqk", name="bqk_t")
            vst = pp.tile([128, totch], F32, tag="vst", name="vst")
            xt = [xt_all[:, c] for c in range(KC)]
            wqs = [wq_all[:, c] for c in range(KC)]
            wks = [wk_all[:, c] for c in range(KC)]
            wvs = [wv_all[:, c] for c in range(KC)]
            wos = [wo_all[:, c] for c in range(KC)]
            bq_t = [bqk_t[:, c:c + 1] for c in range(KC)]
            bk_t = [bqk_t[:, KC + c:KC + c + 1] for c in range(KC)]

            qts = [pp.tile([128, nc_tot], F32R, tag=f"qts{c}", name=f"qts{c}")
                   for c in range(KC)]
            kts = [pp.tile([128, nck], F32R, tag=f"kts{c}", name=f"kts{c}")
                   for c in range(KC)]
            # ots[dc]: [:, 0, :] final (even heads rows 0-63, odd 64-127
            # after shift); [:, 1, :] odd-head staging at rows 0-63.
            # bf16 so the yproj stationary matches wo's dtype.
            ots = [pp.tile([128, 2, nc_tot], BF16, tag=f"ots{c}",
                           name=f"ots{c}") for c in range(KC)]
            ones_t = pp.tile([128, HEAD_DIM], F32R, tag="ones", name="ones")
            v65 = {}
            for g in range(G):
                for mi in range(mg[g]):
                    v65[(g, mi)] = pp.tile([128, NUM_HEADS, HEAD_DIM + 1],
                                           F32R, tag=f"v{g}_{mi}",
                                           name=f"v{g}_{mi}")
            o65all = pp.tile([65, o_total], F32, tag="o65", name="o65all")
            d_all = pp.tile([65, o_total], F32R, tag="dall", name="d_all")
            y_sb = [pp.tile([128, mg[g], HIDDEN], BF16, tag=f"ysb{g}",
                            name=f"ysb{g}") for g in range(G)]

            # ---- input DMAs (3 queues; xT in column halves and wq/wk in
            # output halves; xt-h1 and wq-hA lead separate queues so the
            # first projection group starts ~3us in) ---------------------
            def _wslice(w_all, w, sl):
                return dict(out=w_all[:, :, sl],
                            in_=w[:, sl].rearrange("(c p) n -> p c n",
                                                   p=128))
            hA, hB = slice(0, 256), slice(256, 512)
            nc.sync.dma_start(**_wslice(wq_all, wq, hA))
            (xn0, xnw) = n_tiles_all[0]
            nc.scalar.dma_start(
                out=xt_all[:, :, xn0:xn0 + xnw],
                in_=xT[:, xn0:xn0 + xnw]
                .rearrange("(c p) n -> p c n", p=128))
            nc.sync.dma_start(**_wslice(wk_all, wk, hA))
            for (n0, nw) in n_tiles_all[1:]:
                nc.scalar.dma_start(
                    out=xt_all[:, :, n0:n0 + nw],
                    in_=xT[:, n0:n0 + nw]
                    .rearrange("(c p) n -> p c n", p=128))
            nc.sync.dma_start(
                out=wv_all[:],
                in_=wv[:, :].rearrange("(c p) n -> p c n", p=128))
            nc.sync.dma_start(**_wslice(wq_all, wq, hB))
            nc.sync.dma_start(**_wslice(wk_all, wk, hB))
            nc.sync.dma_start(
                out=wo_all[:],
                in_=wo[:, :].rearrange("(c p) n -> p c n", p=128))
            nc.sync.dma_start(
                out=wo_odd_t[:],
                in_=wo_odd[:, :].rearrange("(j p) n -> p j n", p=64))
            nc.gpsimd.dma_start(out=bqk_t[:],
                                in_=bqk[:].rearrange("b p -> p b"))
            nc.gpsimd.dma_start(out=vst[:],
                                in_=vones[:].rearrange("(k p) -> p k", p=128))

            nc.gpsimd.memset(ones_t[:].bitcast(F32), 1.0)
            warm = vp.tile([1, 2], F32, tag="warm")
            nc.scalar.activation(out=warm[:], in_=ones_t[0:1, 0:2].bitcast(F32),
                                 func=mybir.ActivationFunctionType.Exp,
                                 scale=0.125)
            if pad_tail:
                for c in range(KC):
                    nc.gpsimd.memset(kts[c][:, nc_tot:].bitcast(F32), 0.0)
            for g in range(G):
                _, pm_last = mchunks[g][-1]
                if pm_last < 128:
                    nc.gpsimd.memset(y_sb[g][:, mg[g] - 1, :], 0.0)

            # vones -> ones column of each v65 tile (free-dim broadcast on
            # the otherwise-idle Pool engine)
            for g in range(G):
                for mi in range(mg[g]):
                    src = vst[:, aoff[g] // 128 + mi: aoff[g] // 128 + mi + 1]
                    srcb = bass.AP(tensor=src.tensor, offset=src.offset,
                                   ap=[src.ap[0], [0, NUM_HEADS], [0, 1]])
                    with nc.allow_low_precision(reason="0/1 mask to f32r"):
                        nc.gpsimd.tensor_copy(
                            out=v65[(g, mi)][:, :, HEAD_DIM:], in_=srcb)

            # ---- emit helpers ----------------------------------------
            def emit_qk_group(dst, w_t, bias_t, dc, i, n0, nw, ps, on_act):
                q_ps = ps.tile([128, MAX_FREE], F32, tag="qk")
                for c in range(KC):
                    nc.tensor.matmul(
                        q_ps[:, :nw],
                        w_t[c][:, dc * 128:(dc + 1) * 128],
                        xt[c][:, n0:n0 + nw],
                        start=(c == 0), stop=(c == KC - 1))
                if on_act:
                    nc.scalar.add(out=dst[dc][:, n0:n0 + nw],
                                  in_=q_ps[:, :nw], add=bias_t[dc][:])
                else:
                    with nc.allow_low_precision(reason="f32r eviction"):
                        nc.vector.tensor_scalar_add(dst[dc][:, n0:n0 + nw],
                                                    q_ps[:, :nw],
                                                    bias_t[dc][:])

            def emit_v(g, mi, ps_o):
                m0, pm = mchunks[g][mi]
                a0 = offs[g] + m0
                v_ps = ps_o.tile([128, MAX_FREE], F32, tag="ops")
                for c in range(KC):
                    nc.tensor.matmul(
                        v_ps[:pm, :],
                        xt[c][:, a0:a0 + pm],
                        wvs[c][:],
                        start=(c == 0), stop=(c == KC - 1))
                nc.vector.tensor_copy(
                    out=v65[(g, mi)][:pm, :, :HEAD_DIM],
                    in_=v_ps[:pm, :].rearrange("p (h d) -> p h d",
                                               h=NUM_HEADS))

            def emit_scores(g, h, ps_s):
                dc, r0, g0 = h // 2, (h % 2) * 64, offs[g]
                s = slot_sizes[g]
                # chunk pitch 256 when the slot fits: two chunks share one
                # PSUM bank (each matmul output stays within a bank)
                pitch = 256 if s <= 256 else MAX_FREE
                s_ps = ps_s.tile([128, mg[g], pitch], F32, tag=f"sps{g}")
                for mi, (m0, pm) in enumerate(mchunks[g]):
                    nc.tensor.matmul(
                        s_ps[:, mi, :s],
                        kts[dc][r0:r0 + 64, g0 + m0:g0 + m0 + 128],
                        qts[dc][r0:r0 + 64, g0:g0 + s],
                        start=True, stop=True)
                e_t = ep.tile([128, mg[g], s], F32R, tag=f"e{g}")
                nc.scalar.activation(
                    out=e_t[:, :, :s], in_=s_ps[:, :, :s],
                    func=mybir.ActivationFunctionType.Exp,
                    scale=0.125)
                return e_t

            def emit_av(g, h, e_t, ps_o, copy_act=False, no_copy=False):
                s = slot_sizes[g]
                o_ps = ps_o.tile([65, MAX_FREE], F32, tag="ops")
                for mi, (m0, pm) in enumerate(mchunks[g]):
                    nc.tensor.matmul(
                        o_ps[:, :s],
                        v65[(g, mi)][:pm, h, :],
                        e_t[:pm, mi, :s],
                        start=(mi == 0), stop=(mi == mg[g] - 1))
                if no_copy:
                    return o_ps
                c0 = icol(g, h)
                if copy_act:
                    nc.scalar.copy(out=o65all[:, c0:c0 + s],
                                   in_=o_ps[:, :s])
                else:
                    nc.vector.tensor_copy(out=o65all[:, c0:c0 + s],
                                          in_=o_ps[:, :s])
                return o_ps

            def emit_recip(g, j):
                # head pair j: even|odd columns adjacent
                s = slot_sizes[g]
                c0 = icol(g, 2 * j)
                with nc.allow_low_precision(
                        reason="f32r rounding for PE broadcast"):
                    nc.vector.reciprocal(
                        out=d_all[64:65, c0:c0 + 2 * s],
                        in_=o65all[64:65, c0:c0 + 2 * s])

            def emit_norm(g, h, ps_rb):
                # broadcast 1/d across 64 partitions (PE) then multiply
                dc, g0 = h // 2, offs[g]
                s = slot_sizes[g]
                c0 = icol(g, h)
                rb_ps = ps_rb.tile([64, MAX_FREE], F32, tag="rb")
                nc.tensor.matmul(
                    rb_ps[:, :s],
                    ones_t[64:65, :],
                    d_all[64:65, c0:c0 + s],
                    start=True, stop=True)
                with nc.allow_low_precision(reason="bf16 attention output"):
                    nc.vector.tensor_mul(
                        ots[dc][0:64, h % 2, g0:g0 + s],
                        o65all[0:64, c0:c0 + s], rb_ps[:, :s])

            def emit_norm_pair(g, j, ps_rb):
                # both parities in one DVE multiply: rb holds 1/d for the
                # even|odd halves in two banks; the 3D output view writes
                # the even half and the odd staging of the same ots tile
                dc, g0 = j, offs[g]
                s = slot_sizes[g]
                c0 = icol(g, 2 * j)
                pitch = 256 if s <= 256 else MAX_FREE
                rb2 = ps_rb.tile([64, 2, pitch], F32, tag="rbp")
                if s == pitch:
                    # contiguous pair: one broadcast matmul covers both
                    nc.tensor.matmul(
                        rb2[:, :, :].rearrange("p a b -> p (a b)"),
                        ones_t[64:65, :],
                        d_all[64:65, c0:c0 + 2 * s],
                        start=True, stop=True)
                else:
                    for par in range(2):
                        nc.tensor.matmul(
                            rb2[:, par, :s],
                            ones_t[64:65, :],
                            d_all[64:65, c0 + par * s:c0 + par * s + s],
                            start=True, stop=True)
                with nc.allow_low_precision(reason="bf16 attention output"):
                    nc.vector.tensor_mul(
                        ots[dc][0:64, :, g0:g0 + s],
                        o65all[0:64, c0:c0 + 2 * s]
                        .rearrange("p (j w) -> p j w", j=2),
                        rb2[:, :, :s])

            def emit_recip_act(g, j):
                # 1/d = exp(-ln d) on ACT: both functions live in the
                # natural_log_exp_and_others table set (no switch)
                s = slot_sizes[g]
                c0 = icol(g, 2 * j)
                lnd = vp.tile([65, 2 * MAX_FREE], F32, tag="lnd")
                with nc.allow_low_precision(reason="recip via ln/exp"):
                    nc.scalar.activation(
                        out=lnd[64:65, :2 * s],
                        in_=o65all[64:65, c0:c0 + 2 * s],
                        func=mybir.ActivationFunctionType.Ln)
                    nc.scalar.activation(
                        out=d_all[64:65, c0:c0 + 2 * s].bitcast(F32),
                        in_=lnd[64:65, :2 * s],
                        func=mybir.ActivationFunctionType.Exp,
                        scale=-1.0)

            def emit_recip_direct(g, h, o_ps):
                # reciprocal straight from the AV psum's denominator row
                s = slot_sizes[g]
                c0 = icol(g, h)
                with nc.allow_low_precision(
                        reason="f32r rounding for PE broadcast"):
                    nc.vector.reciprocal(
                        out=d_all[64:65, c0:c0 + s],
                        in_=o_ps[64:65, :s])

            def emit_norm_direct(g, h, o_ps, ps_rb):
                # normalize straight from psum (no o65all staging copy)
                dc, g0 = h // 2, offs[g]
                s = slot_sizes[g]
                c0 = icol(g, h)
                rb_ps = ps_rb.tile([64, MAX_FREE], F32, tag="rb")
                nc.tensor.matmul(
                    rb_ps[:, :s],
                    ones_t[64:65, :],
                    d_all[64:65, c0:c0 + s],
                    start=True, stop=True)
                with nc.allow_low_precision(reason="bf16 attention output"):
                    nc.vector.tensor_mul(
                        ots[dc][0:64, h % 2, g0:g0 + s],
                        o_ps[0:64, :s], rb_ps[:, :s])

            def emit_shift(g, dc):
                # odd-head staging -> partition base 64 (DMA only); on the
                # Pool/SWDGE queue so y DMAs never queue ahead of it
                g0, gw = offs[g], slot_sizes[g]
                nc.gpsimd.dma_start(out=ots[dc][64:128, 0, g0:g0 + gw],
                                    in_=ots[dc][0:64, 1, g0:g0 + gw])

            def emit_yproj_mm(g, mi, ps_y, split_from=KC):
                m0, pm = mchunks[g][mi]
                a0 = offs[g] + m0
                y_ps = ps_y.tile([128, MAX_FREE], F32, tag="y")
                for dc in range(split_from):
                    nc.tensor.matmul(
                        y_ps[:pm, :],
                        ots[dc][:, 0, a0:a0 + pm],
                        wos[dc][:],
                        start=(dc == 0), stop=(dc == KC - 1))
                for dc in range(split_from, KC):
                    # read the odd-head staging directly (partitions 0-63)
                    # instead of waiting for that pair's shift DMA
                    nc.tensor.matmul(
                        y_ps[:pm, :],
                        ots[dc][0:64, 0, a0:a0 + pm],
                        wos[dc][0:64, :],
                        start=False, stop=False)
                    nc.tensor.matmul(
                        y_ps[:pm, :],
                        ots[dc][0:64, 1, a0:a0 + pm],
                        wo_odd_t[:, dc - 2, :],
                        start=False, stop=(dc == KC - 1))
                return y_ps

            def emit_yevict(g, mi, y_ps, evict_dve=False, split=False):
                _, pm = mchunks[g][mi]
                with nc.allow_low_precision(reason="bf16 output staging"):
                    if split:
                        # halves on both engines in parallel (tail latency)
                        nc.scalar.copy(out=y_sb[g][:pm, mi, :256],
                                       in_=y_ps[:pm, :256])
                        nc.vector.tensor_copy(out=y_sb[g][:pm, mi, 256:],
                                              in_=y_ps[:pm, 256:])
                    elif evict_dve:
                        nc.vector.tensor_copy(out=y_sb[g][:pm, mi, :],
                                              in_=y_ps[:pm, :])
                    else:
                        nc.scalar.copy(out=y_sb[g][:pm, mi, :],
                                       in_=y_ps[:pm, :])

            def emit_yproj(g, mi, ps_y, split_from=KC, evict_dve=False,
                           evict_split=False):
                y_ps = emit_yproj_mm(g, mi, ps_y, split_from)
                emit_yevict(g, mi, y_ps, evict_dve, split=evict_split)

            def emit_yproj_tail(g, mi, ps_y, split_from):
                # last chunk: column-halved matmuls/evicts/DMAs so the two
                # evictions run on ACT||DVE and the first half-DMA issues
                # while the second half is still projecting
                m0, pm = mchunks[g][mi]
                a0 = offs[g] + m0
                y_ps2 = ps_y.tile([128, 2, 256], F32, tag="y")
                for half in range(2):
                    cs = slice(half * 256, half * 256 + 256)
                    y_ps = y_ps2[:, half]
                    for dc in range(split_from):
                        nc.tensor.matmul(
                            y_ps[:pm, :],
                            ots[dc][:, 0, a0:a0 + pm],
                            wos[dc][:, cs],
                            start=(dc == 0), stop=(dc == KC - 1))
                    for dc in range(split_from, KC):
                        nc.tensor.matmul(
                            y_ps[:pm, :],
                            ots[dc][0:64, 0, a0:a0 + pm],
                            wos[dc][0:64, cs],
                            start=False, stop=False)
                        nc.tensor.matmul(
                            y_ps[:pm, :],
                            ots[dc][0:64, 1, a0:a0 + pm],
                            wo_odd_t[:, dc - 2, cs],
                            start=False, stop=(dc == KC - 1))
                    with nc.allow_low_precision(reason="bf16 staging"):
                        if half == 0:
                            nc.scalar.copy(out=y_sb[g][:pm, mi, cs],
                                           in_=y_ps[:pm, :])
                        else:
                            nc.vector.tensor_copy(out=y_sb[g][:pm, mi, cs],
                                                  in_=y_ps[:pm, :])
                    dst = y[aoff[g] + mi * 128:aoff[g] + (mi + 1) * 128, cs]
                    nc.sync.dma_start(out=dst, in_=y_sb[g][:, mi, cs])

            def emit_ydma(g, mi=None):
                if mi is None:
                    dst = y[aoff[g]:aoff[g] + mg[g] * 128, :]
                    nc.sync.dma_start(
                        out=dst.rearrange("(m p) n -> p m n", p=128),
                        in_=y_sb[g][:])
                else:
                    dst = y[aoff[g] + mi * 128:aoff[g] + (mi + 1) * 128, :]
                    nc.sync.dma_start(out=dst, in_=y_sb[g][:, mi])

            def run_slot(g, ps_s, ps_o, ps_rb, inject, start_av=2,
                         copy_act_odd=False, split_from=KC,
                         direct_norm=False, recip_act=False,
                         copy_act_all=False, merge_mul=False,
                         merge_exp=False, recip_psum=False):
                """Emit slot g's pipelined item stream.

                inject: dict item_index -> list of thunks to emit there
                (e.g. QK projection groups, V projections, prev slot's
                yproj). Engines execute their queues in order, so this
                emission order IS the per-engine execution order.
                """
                pend = []          # (h, e_t) awaiting AV
                done_pairs = []    # pairs awaiting norm emission
                o_tiles = {}       # h -> AV psum tile (direct_norm)
                norm_pend = []     # pairs with recip emitted, mul pending
                av_done = 0

                def pop_av():
                    nonlocal av_done
                    h, e_t = pend.pop(0)
                    o_tiles[h] = emit_av(
                        g, h, e_t, ps_o, no_copy=direct_norm,
                        copy_act=copy_act_all
                        or (copy_act_odd and h % 2 == 1))
                    av_done += 1
                    if av_done % 2 == 0:
                        done_pairs.append(av_done // 2 - 1)

                def pop_norm():
                    j = done_pairs.pop(0)
                    if direct_norm:
                        emit_recip_direct(g, 2 * j, o_tiles[2 * j])
                        emit_recip_direct(g, 2 * j + 1, o_tiles[2 * j + 1])
                        emit_norm_direct(g, 2 * j, o_tiles[2 * j], ps_rb)
                        emit_norm_direct(g, 2 * j + 1, o_tiles[2 * j + 1],
                                         ps_rb)
                    else:
                        if recip_act:
                            emit_recip_act(g, j)
                        elif recip_psum:
                            # straight from the AV psum denominator rows:
                            # runs concurrently with the o65 staging copy
                            emit_recip_direct(g, 2 * j, o_tiles[2 * j])
                            emit_recip_direct(g, 2 * j + 1,
                                              o_tiles[2 * j + 1])
                        else:
                            emit_recip(g, j)
                        if merge_mul:
                            emit_norm_pair(g, j, ps_rb)
                        else:
                            emit_norm(g, 2 * j, ps_rb)
                            emit_norm(g, 2 * j + 1, ps_rb)
                    if j < split_from:
                        emit_shift(g, j)

                s = slot_sizes[g]
                pair_ctx = [None]

                def scores_item(h):
                    if not merge_exp:
                        return emit_scores(g, h, ps_s)
                    # pair-merged exp: both heads' scores in one contiguous
                    # 2-bank tile, ONE activation covers the pair
                    if h % 2 == 0:
                        sps = ps_s.tile([128, 2, mg[g], 256], F32,
                                        tag=f"sps{g}")
                        et = ep.tile([128, 2, mg[g], s], F32R, tag=f"e{g}")
                        pair_ctx[0] = (sps, et)
                    sps, et = pair_ctx[0]
                    dc, r0, g0 = h // 2, (h % 2) * 64, offs[g]
                    for mi, (m0, pm) in enumerate(mchunks[g]):
                        nc.tensor.matmul(
                            sps[:, h % 2, mi, :s],
                            kts[dc][r0:r0 + 64, g0 + m0:g0 + m0 + 128],
                            qts[dc][r0:r0 + 64, g0:g0 + s],
                            start=True, stop=True)
                    if h % 2 == 1:
                        nc.scalar.activation(
                            out=et[:, :, :, :]
                            .rearrange("p a b c -> p (a b c)"),
                            in_=sps[:, :, :, :]
                            .rearrange("p a b c -> p (a b c)"),
                            func=mybir.ActivationFunctionType.Exp,
                            scale=0.125)
                    return et[:, h % 2]

                for h in range(NUM_HEADS):
                    for th in inject.get(h, ()):
                        th()
                    pend.append((h, scores_item(h)))
                    if h >= start_av:
                        pop_av()
                    if h >= start_av + 2 and done_pairs:
                        pop_norm()
                for th in inject.get(NUM_HEADS, ()):
                    th()
                while pend:
                    pop_av()
                    if done_pairs:
                        pop_norm()
                while done_pairs:
                    pop_norm()
                while norm_pend:
                    jj = norm_pend.pop(0)
                    emit_norm_pair(g, jj, ps_rb)
                    if jj < split_from:
                        emit_shift(g, jj)

            # ---- emission ---------------------------------------------
            # PSUM banks: qk(2) + s0(mg0) + o(2) + rb(1) <= 8 during the
            # front phase; s1(mg1) + y(2) + o(2) + rb(1) <= 8 in the back.
            assert mg[0] + 5 <= 8, f"slot0 too large: {mg[0]} score banks"
            if G == 2:
                assert mg[1] + 5 <= 8, f"slot1 too large: {mg[1]} banks"
            with tc.tile_pool(name="ps_o", bufs=2, space="PSUM") as ps_o:
                with (
                    tc.tile_pool(name="ps_qk", bufs=2, space="PSUM") as ps_qk,
                    tc.tile_pool(name="ps_s0", bufs=1, space="PSUM") as ps_s0,
                    tc.tile_pool(name="ps_rb", bufs=1, space="PSUM") as ps_rb,
                ):
                    # slot0 items interleaved with QK projection groups:
                    # heads 2dc..2dc+1 follow the dc they depend on.
                    inj = {}
                    for dc in range(KC):
                        th = []
                        for i, (n0, nw) in enumerate(n_tiles_all):
                            th.append(lambda dc=dc, n0=n0, nw=nw:
                                      emit_qk_group(qts, wqs, bq_t, dc, 0,
                                                    n0, nw, ps_qk, True))
                            th.append(lambda dc=dc, n0=n0, nw=nw:
                                      emit_qk_group(kts, wks, bk_t, dc, 0,
                                                    n0, nw, ps_qk, False))
                        inj[2 * dc] = th
                    # V projections at h=4..7: by then the PE pstate ramp
                    # is warm (these are 512-wide moving operands)
                    for i in range(mg[0]):
                        inj.setdefault(2, []).append(
                            lambda mi=i: emit_v(0, mi, ps_o))
                    if G == 2:
                        for i in range(mg[1]):
                            inj.setdefault(4 + min(i, 1), []).append(
                                lambda mi=i: emit_v(1, mi, ps_o))
                    run_slot(0, ps_s0, ps_o, ps_rb, inj, start_av=4,
                             copy_act_odd=True,
                             split_from=(2 if G == 1 else KC))

                if G == 1:
                    with tc.tile_pool(name="ps_y", bufs=2,
                                      space="PSUM") as ps_y:
                        for mi in range(mg[0]):
                            emit_yproj(0, mi, ps_y, split_from=2,
                                       evict_dve=(mi % 2 == 1))
                            emit_ydma(0, mi)
                else:
                    with (
                        tc.tile_pool(name="ps_s1", bufs=1,
                                     space="PSUM") as ps_s1,
                        tc.tile_pool(name="ps_y", bufs=2,
                                     space="PSUM") as ps_y,
                        tc.tile_pool(name="ps_rb2", bufs=2,
                                     space="PSUM") as ps_rb2,
                    ):
                        def yp0(mi):
                            emit_yproj(0, mi, ps_y)
                            emit_ydma(0, mi)

                        inj = {5 + i: [lambda mi=mi: yp0(mi)]
                               for i, mi in enumerate(range(mg[0]))}
                        run_slot(1, ps_s1, ps_o, ps_rb2, inj,
                                 copy_act_all=True, split_from=2,
                                 merge_mul=True,
                                 merge_exp=(slot_sizes[1] <= 256))
                        for mi in range(mg[1]):
                            emit_yproj(1, mi, ps_y, split_from=2,
                                       evict_dve=(mi % 2 == 0))
                            emit_ydma(1, mi)

    nc.compile()
    return nc


def _plan(batch):
    """Assign whole graphs (contiguous segments) to cores/slots.

    Returns (slot_sizes, assign) where assign[core][slot] = (start, size)
    of the graph segment in the global node order (size 0 = empty slot).
    """
    batch = np.asarray(batch)
    vals, starts, counts = np.unique(batch, return_index=True,
                                     return_counts=True)
    segs = sorted(zip(starts.tolist(), counts.tolist()),
                  key=lambda t: -t[1])
    n_slots = _ceil_div(len(segs), N_CORES)
    while len(segs) < n_slots * N_CORES:
        segs.append((0, 0))
    assign = [[None] * n_slots for _ in range(N_CORES)]
    slot_sizes = []
    for j in range(n_slots):
        block = segs[j * N_CORES:(j + 1) * N_CORES]
        order = range(N_CORES) if j % 2 == 0 else range(N_CORES - 1, -1, -1)
        for c, k in zip(order, range(N_CORES)):
            assign[c][j] = block[k]
        m = max(sz for (_, sz) in block)
        m = _ceil_div(m, 4) * 4
        # f32r matmuls drop to 4 cycles/row below a 256-wide moving free
        # dim — pad mid-sized slots up to 256 to stay on the fast path
        if m >= 64:
            m = max(m, 256)
        slot_sizes.append(m)
    keep = [j for j, s in enumerate(slot_sizes) if s > 0]
    slot_sizes = [slot_sizes[j] for j in keep]
    assign = [[assign[c][j] for j in keep] for c in range(N_CORES)]
    return tuple(slot_sizes), assign


def kernel(x, batch, Wq, bq, Wk, bk, Wv, bv, Wo, bo):
    out, _ = _execute(dict(x=x, batch=batch, Wq=Wq, bq=bq, Wk=Wk, bk=bk,
                           Wv=Wv, bv=bv, Wo=Wo, bo=bo))
    return out


def _aligned_offsets(slot_sizes):
    aoff = [0]
    for s in slot_sizes:
        aoff.append(aoff[-1] + _ceil_div(s, 128) * 128)
    return aoff


def _prepare(inputs):
    x = np.ascontiguousarray(np.asarray(inputs["x"], dtype=np.float32))
    Wq = np.asarray(inputs["Wq"], dtype=np.float32)
    Wk = np.asarray(inputs["Wk"], dtype=np.float32)
    Wv = np.asarray(inputs["Wv"], dtype=np.float32)
    Wo = np.asarray(inputs["Wo"], dtype=np.float32)
    bq = np.asarray(inputs["bq"], dtype=np.float32)
    bk = np.asarray(inputs["bk"], dtype=np.float32)
    bv = np.asarray(inputs["bv"], dtype=np.float32)
    bo = np.asarray(inputs["bo"], dtype=np.float32)

    slot_sizes, assign = _plan(inputs["batch"])
    offs = np.concatenate([[0], np.cumsum(slot_sizes)]).astype(int)
    nc_tot = int(offs[-1])
    aoff = _aligned_offsets(slot_sizes)

    bf = ml_dtypes.bfloat16
    wqT = np.ascontiguousarray(Wq.T).astype(bf)
    wkT = np.ascontiguousarray(Wk.T).astype(bf)
    wvT = np.ascontiguousarray(Wv.T).astype(bf)
    woT = np.ascontiguousarray(Wo.T).astype(bf)
    # V-bias and out-bias fold: softmax rows sum to 1, so attn@(V+bv) =
    # attn@V + bv, and (O+bv)@Wo.T + bo = O@Wo.T + (Wo@bv + bo).
    b_out = (Wo @ bv + bo).astype(np.float32)
    bqk = np.concatenate([bq.reshape(KC, 128), bk.reshape(KC, 128)], axis=0)
    bqk = np.ascontiguousarray(bqk)

    in_maps = []
    for c in range(N_CORES):
        xT = np.zeros((HIDDEN, nc_tot), dtype=np.float32)
        von = np.zeros((aoff[-1],), dtype=np.float32)
        for j, (st, sz) in enumerate(assign[c]):
            if sz:
                xT[:, offs[j]:offs[j] + sz] = x[st:st + sz].T
                von[aoff[j]:aoff[j] + sz] = 1.0
        in_maps.append({
            "xT": xT.astype(bf), "wq": wqT, "wk": wkT, "wv": wvT, "wo": woT,
            "wo_odd": np.ascontiguousarray(
                np.concatenate([woT[320:384, :], woT[448:512, :]], axis=0)),
            "bqk": bqk, "vones": von,
        })
    return slot_sizes, assign, offs, in_maps, b_out


def _gather(results, assign, offs, n_nodes, b_out, slot_sizes):
    aoff = _aligned_offsets(slot_sizes)
    out = np.empty((n_nodes, HIDDEN), dtype=np.float32)
    for c in range(N_CORES):
        yc = np.asarray(results[c]["y"])
        if yc.dtype != np.float32:
            yc = yc.view(ml_dtypes.bfloat16).astype(np.float32) \
                if yc.dtype.itemsize == 2 else yc.astype(np.float32)
        else:
            yc = yc
        for j, (st, sz) in enumerate(assign[c]):
            if sz:
                out[st:st + sz] = yc[aoff[j]:aoff[j] + sz]
    out += b_out[None, :]
    return out


def _execute(inputs, trace=False, **run_kwargs):
    slot_sizes, assign, offs, in_maps, b_out = _prepare(inputs)
    if slot_sizes not in _CACHE:
        _CACHE[slot_sizes] = _build_program(list(slot_sizes))
    nc = _CACHE[slot_sizes]
    res = run_bass_kernel_spmd(nc, in_maps, list(range(N_CORES)),
                               trace=trace, **run_kwargs)
    out = _gather(res.results, assign, offs,
                  np.asarray(inputs["x"]).shape[0], b_out, slot_sizes)
    return out, res


# revision 114
# speedup vs baseline: 1.0173x; 1.0173x over previous
"""Trainium2 Bass kernel for block-diagonal (per-graph) multi-head attention.

Full inputs in, full output out. Host side: graphs (contiguous segments of
the sorted node dim) are assigned whole to 8 NeuronCores (2 padded slots per
core, boustrophedon by size), weights replicated, x pre-transposed; outputs
are gathered back and the foldable biases (bv via softmax-rows-sum-to-1, bo)
are applied on the host.

Device program (SPMD, one compiled program, per-core data). Key structure
(v2 — overlap-oriented rewrite of the phase-sequential baseline; emission
order == per-engine execution order since engines execute their queues
in order, so the item pipeline is interleaved at emission time):
  - consolidated DMAs (HWDGE fixed cost is per instruction): xT in two
    slot-aligned column-halves on the ACT queue, wq/wk in output halves +
    wv/wo whole on the SP queue, small tensors on the Pool/SWDGE queue;
    wo and y in bf16 to halve bytes
  - the per-node 0/1 "vones" mask (excludes padded keys from both the AV
    numerator and the softmax denominator) is ONE row DMA + tiny Pool
    free-dim-broadcast copies into the packed V tiles' 65th column
  - front phase, all in one PSUM scope (qk 2 banks with per-slot-tile
    eviction ping-pong + scores0 mg0 + AV/V 2 + recip-broadcast 1):
    QK projection groups interleaved with slot0's scores/exp/AV/norm
    item pipeline; V projections injected mid-stream once wv lands
  - per (slot, head): scores^T -> ONE merged exp on ACT (scale=1/8) ->
    AV with the ones-column emitting the softmax denominator as row 64
  - normalization pipelined into each slot's pass: pair-merged
    reciprocals ([1, 2s] over head-pair-adjacent columns of o65all),
    PE-broadcast of 1/d across 64 partitions, per-item multiply into
    bf16 ots (odd heads into a staging half of the same tile, then one
    SBUF->SBUF DMA shift to partition base 64 on the Pool queue —
    compute engines cannot shift partitions)
  - back phase (scores1 + yproj 2 + recip-broadcast 2 banks): slot1's
    pipeline with slot0's per-chunk output projections + y DMAs
    injected; slot1 skips the shift for its last two head pairs — their
    yproj reads the odd staging directly via a 64-row contraction split
    against an extra copy of wo's odd rows (wo_odd)
  - engine balance: Q evictions + slot1-odd o65 copies + y evictions on
    ACT; K evictions + V copies + o65 copies + reciprocals + normalize
    muls on DVE; vones broadcasts + shifts on Pool
  - output projections per 128-node chunk, staged to bf16 and DMAd per
    chunk into a 128-aligned y layout
"""

import os
import sys

import ml_dtypes
import numpy as np

for _p in ("/opt/trn_rl_repo", os.path.expanduser("~/.axon_site/_ro/trn_rl_repo")):
    if os.path.isdir(_p) and _p not in sys.path:
        sys.path.insert(0, _p)

import concourse.bacc as bacc
import concourse.bass as bass
import concourse.mybir as mybir
import concourse.tile as tile
from concourse.bass_utils import run_bass_kernel_spmd

N_CORES = 8
HIDDEN = 512
NUM_HEADS = 8
HEAD_DIM = 64
KC = HIDDEN // 128  # contraction chunks of 128
F32 = mybir.dt.float32
F32R = mybir.dt.float32r
BF16 = mybir.dt.bfloat16
MAX_FREE = 512  # psum bank limit for fp32 free dim

_CACHE: dict = {}


def _ceil_div(a, b):
    return -(-a // b)


def _split_free(n, max_w=MAX_FREE):
    """Split even n into nearly equal EVEN pieces <= max_w (f32r matmuls
    need an even moving free dim)."""
    assert n % 2 == 0, n
    h = n // 2
    k = _ceil_div(n, max_w)
    base = h // k
    rem = h - base * k
    out = []
    off = 0
    for i in range(k):
        w = 2 * (base + (1 if i < rem else 0))
        out.append((off, w))
        off += w
    return out


def _build_program(slot_sizes):
    """Build + compile the SPMD Bass program for padded slot sizes."""
    G = len(slot_sizes)
    offs = [0]
    for s in slot_sizes:
        offs.append(offs[-1] + s)
    nc_tot = offs[-1]

    mchunks = {}
    for g in range(G):
        s = slot_sizes[g]
        mchunks[g] = [(mi * 128, min(128, s - mi * 128))
                      for mi in range(_ceil_div(s, 128))]
    mg = {g: len(mchunks[g]) for g in range(G)}
    # K columns are read in full-128 chunks by the scores matmuls; zero-pad
    pad_tail = max(0, _ceil_div(slot_sizes[-1], 128) * 128 - slot_sizes[-1])
    nck = nc_tot + pad_tail

    # 128-aligned per-slot offsets (for vones staging and the y layout)
    aoff = [0]
    for g in range(G):
        aoff.append(aoff[-1] + mg[g] * 128)
    tot_al = aoff[-1]
    totch = tot_al // 128

    # o65all column layout: pair-major so a head-pair's (even|odd) columns
    # are adjacent -> one reciprocal per pair
    gcol = {}
    oc = 0
    for g in range(G):
        gcol[g] = oc
        oc += 2 * NUM_HEADS // 2 * slot_sizes[g]
    o_total = oc

    def icol(g, h):
        return gcol[g] + (h // 2) * 2 * slot_sizes[g] + (h % 2) * slot_sizes[g]

    # Q/K eviction tiles aligned to slot boundaries: the first slot's
    # scores then depend only on the first tile's eviction
    if max(slot_sizes) <= MAX_FREE:
        n_tiles_all = [(offs[g], slot_sizes[g]) for g in range(G)]
    else:
        n_tiles_all = _split_free(nc_tot)

    nc = bacc.Bacc("TRN2", target_bir_lowering=False, debug=False,
                   num_devices=N_CORES)

    xT = nc.dram_tensor("xT", [HIDDEN, nc_tot], BF16, kind="ExternalInput")
    # wq/wk host-packed partition-major per output chunk: [p, dc, c, n]
    wq = nc.dram_tensor("wq", [128, KC * HIDDEN], BF16, kind="ExternalInput")
    wk = nc.dram_tensor("wk", [128, KC * HIDDEN], BF16, kind="ExternalInput")
    wv = nc.dram_tensor("wv", [HIDDEN, HIDDEN], BF16, kind="ExternalInput")
    wo = nc.dram_tensor("wo", [HIDDEN, HIDDEN], BF16, kind="ExternalInput")
    # odd-head rows of wo's dc2/dc3 chunks again, loaded at partitions
    # 0-63: lets the last head-pairs' yproj read the odd-head staging
    # (partitions 0-63) without waiting for the partition-shift DMA
    wo_odd = nc.dram_tensor("wo_odd", [128, HIDDEN], BF16,
                            kind="ExternalInput")
    bqk = nc.dram_tensor("bqk", [2 * KC, 128], F32, kind="ExternalInput")
    vones = nc.dram_tensor("vones", [tot_al], F32, kind="ExternalInput")
    y = nc.dram_tensor("y", [tot_al, HIDDEN], BF16, kind="ExternalOutput")

    with tile.TileContext(nc) as tc:
        with (
            tc.tile_pool(name="persist", bufs=1) as pp,
            tc.tile_pool(name="ework", bufs=8) as ep,
            tc.tile_pool(name="vwork", bufs=3) as vp,
        ):
            # ---- persistent tiles ------------------------------------
            xt_all = pp.tile([128, KC, nc_tot], BF16, tag="xt", name="xt")
            wq_all = pp.tile([128, KC, KC, 128], BF16, tag="wq",
                             name="wq_t")
            wk_all = pp.tile([128, KC, KC, 128], BF16, tag="wk",
                             name="wk_t")
            wv_all = pp.tile([128, KC, HIDDEN], BF16, tag="wv", name="wv_t")
            wo_all = pp.tile([128, KC, HIDDEN], BF16, tag="wo", name="wo_t")
            wo_odd_t = pp.tile([64, 2, HIDDEN], BF16, tag="wood",
                               name="wo_odd")
            bqk_t = pp.tile([128, 2 * KC], F32, tag="bqk", name="bqk_t")
            vst = pp.tile([128, totch], F32, tag="vst", name="vst")
            xt = [xt_all[:, c] for c in range(KC)]
            wvs = [wv_all[:, c] for c in range(KC)]
            wos = [wo_all[:, c] for c in range(KC)]
            bq_t = [bqk_t[:, c:c + 1] for c in range(KC)]
            bk_t = [bqk_t[:, KC + c:KC + c + 1] for c in range(KC)]

            qts = [pp.tile([128, nc_tot], F32R, tag=f"qts{c}", name=f"qts{c}")
                   for c in range(KC)]
            kts = [pp.tile([128, nck], F32R, tag=f"kts{c}", name=f"kts{c}")
                   for c in range(KC)]
            # ots[dc]: [:, 0, :] final (even heads rows 0-63, odd 64-127
            # after shift); [:, 1, :] odd-head staging at rows 0-63.
            # bf16 so the yproj stationary matches wo's dtype.
            ots = [pp.tile([128, 2, nc_tot], BF16, tag=f"ots{c}",
                           name=f"ots{c}") for c in range(KC)]
            ones_t = pp.tile([128, HEAD_DIM], F32R, tag="ones", name="ones")
            v65 = {}
            for g in range(G):
                for mi in range(mg[g]):
                    v65[(g, mi)] = pp.tile([128, NUM_HEADS, HEAD_DIM + 1],
                                           F32R, tag=f"v{g}_{mi}",
                                           name=f"v{g}_{mi}")
            o65all = pp.tile([65, o_total], F32, tag="o65", name="o65all")
            d_all = pp.tile([65, o_total], F32R, tag="dall", name="d_all")
            y_sb = [pp.tile([128, mg[g], HIDDEN], BF16, tag=f"ysb{g}",
                            name=f"ysb{g}") for g in range(G)]

            # ---- input DMAs (3 queues; xT in column halves and wq/wk in
            # output halves; xt-h1 and wq-hA lead separate queues so the
            # first projection group starts ~3us in) ---------------------
            def _wdc(w_all, w, dc):
                return dict(
                    out=w_all[:, dc],
                    in_=w[:, dc * HIDDEN:(dc + 1) * HIDDEN]
                    .rearrange("p (c n) -> p c n", c=KC))
            nc.sync.dma_start(**_wdc(wq_all, wq, 0))
            (xn0, xnw) = n_tiles_all[0]
            nc.scalar.dma_start(
                out=xt_all[:, :, xn0:xn0 + xnw],
                in_=xT[:, xn0:xn0 + xnw]
                .rearrange("(c p) n -> p c n", p=128))
            nc.sync.dma_start(**_wdc(wk_all, wk, 0))
            nc.sync.dma_start(**_wdc(wq_all, wq, 1))
            nc.sync.dma_start(**_wdc(wk_all, wk, 1))
            for (n0, nw) in n_tiles_all[1:]:
                nc.scalar.dma_start(
                    out=xt_all[:, :, n0:n0 + nw],
                    in_=xT[:, n0:n0 + nw]
                    .rearrange("(c p) n -> p c n", p=128))
            nc.sync.dma_start(
                out=wv_all[:],
                in_=wv[:, :].rearrange("(c p) n -> p c n", p=128))
            for dc in (2, 3):
                nc.sync.dma_start(**_wdc(wq_all, wq, dc))
                nc.sync.dma_start(**_wdc(wk_all, wk, dc))
            nc.sync.dma_start(
                out=wo_all[:],
                in_=wo[:, :].rearrange("(c p) n -> p c n", p=128))
            nc.sync.dma_start(
                out=wo_odd_t[:],
                in_=wo_odd[:, :].rearrange("(j p) n -> p j n", p=64))
            nc.gpsimd.dma_start(out=bqk_t[:],
                                in_=bqk[:].rearrange("b p -> p b"))
            nc.gpsimd.dma_start(out=vst[:],
                                in_=vones[:].rearrange("(k p) -> p k", p=128))

            nc.gpsimd.memset(ones_t[:].bitcast(F32), 1.0)
            warm = vp.tile([1, 2], F32, tag="warm")
            nc.scalar.activation(out=warm[:], in_=ones_t[0:1, 0:2].bitcast(F32),
                                 func=mybir.ActivationFunctionType.Exp,
                                 scale=0.125)
            if pad_tail:
                for c in range(KC):
                    nc.gpsimd.memset(kts[c][:, nc_tot:].bitcast(F32), 0.0)
            for g in range(G):
                _, pm_last = mchunks[g][-1]
                if pm_last < 128:
                    nc.gpsimd.memset(y_sb[g][:, mg[g] - 1, :], 0.0)

            # vones -> ones column of each v65 tile (free-dim broadcast on
            # the otherwise-idle Pool engine)
            for g in range(G):
                for mi in range(mg[g]):
                    src = vst[:, aoff[g] // 128 + mi: aoff[g] // 128 + mi + 1]
                    srcb = bass.AP(tensor=src.tensor, offset=src.offset,
                                   ap=[src.ap[0], [0, NUM_HEADS], [0, 1]])
                    with nc.allow_low_precision(reason="0/1 mask to f32r"):
                        nc.gpsimd.tensor_copy(
                            out=v65[(g, mi)][:, :, HEAD_DIM:], in_=srcb)

            # ---- emit helpers ----------------------------------------
            def emit_qk_group(dst, w_t, bias_t, dc, i, n0, nw, ps, on_act):
                q_ps = ps.tile([128, MAX_FREE], F32, tag="qk")
                for c in range(KC):
                    nc.tensor.matmul(
                        q_ps[:, :nw],
                        w_t[:, dc, c, :],
                        xt[c][:, n0:n0 + nw],
                        start=(c == 0), stop=(c == KC - 1))
                if on_act:
                    nc.scalar.add(out=dst[dc][:, n0:n0 + nw],
                                  in_=q_ps[:, :nw], add=bias_t[dc][:])
                else:
                    with nc.allow_low_precision(reason="f32r eviction"):
                        nc.vector.tensor_scalar_add(dst[dc][:, n0:n0 + nw],
                                                    q_ps[:, :nw],
                                                    bias_t[dc][:])

            def emit_v(g, mi, ps_o):
                m0, pm = mchunks[g][mi]
                a0 = offs[g] + m0
                v_ps = ps_o.tile([128, MAX_FREE], F32, tag="ops")
                for c in range(KC):
                    nc.tensor.matmul(
                        v_ps[:pm, :],
                        xt[c][:, a0:a0 + pm],
                        wvs[c][:],
                        start=(c == 0), stop=(c == KC - 1))
                nc.vector.tensor_copy(
                    out=v65[(g, mi)][:pm, :, :HEAD_DIM],
                    in_=v_ps[:pm, :].rearrange("p (h d) -> p h d",
                                               h=NUM_HEADS))

            def emit_scores(g, h, ps_s):
                dc, r0, g0 = h // 2, (h % 2) * 64, offs[g]
                s = slot_sizes[g]
                # chunk pitch 256 when the slot fits: two chunks share one
                # PSUM bank (each matmul output stays within a bank)
                pitch = 256 if s <= 256 else MAX_FREE
                s_ps = ps_s.tile([128, mg[g], pitch], F32, tag=f"sps{g}")
                for mi, (m0, pm) in enumerate(mchunks[g]):
                    nc.tensor.matmul(
                        s_ps[:, mi, :s],
                        kts[dc][r0:r0 + 64, g0 + m0:g0 + m0 + 128],
                        qts[dc][r0:r0 + 64, g0:g0 + s],
                        start=True, stop=True)
                e_t = ep.tile([128, mg[g], s], F32R, tag=f"e{g}")
                nc.scalar.activation(
                    out=e_t[:, :, :s], in_=s_ps[:, :, :s],
                    func=mybir.ActivationFunctionType.Exp,
                    scale=0.125)
                return e_t

            def emit_av(g, h, e_t, ps_o, copy_act=False, no_copy=False):
                s = slot_sizes[g]
                o_ps = ps_o.tile([65, MAX_FREE], F32, tag="ops")
                for mi, (m0, pm) in enumerate(mchunks[g]):
                    nc.tensor.matmul(
                        o_ps[:, :s],
                        v65[(g, mi)][:pm, h, :],
                        e_t[:pm, mi, :s],
                        start=(mi == 0), stop=(mi == mg[g] - 1))
                if no_copy:
                    return o_ps
                c0 = icol(g, h)
                if copy_act:
                    nc.scalar.copy(out=o65all[:, c0:c0 + s],
                                   in_=o_ps[:, :s])
                else:
                    nc.vector.tensor_copy(out=o65all[:, c0:c0 + s],
                                          in_=o_ps[:, :s])
                return o_ps

            def emit_recip(g, j):
                # head pair j: even|odd columns adjacent
                s = slot_sizes[g]
                c0 = icol(g, 2 * j)
                with nc.allow_low_precision(
                        reason="f32r rounding for PE broadcast"):
                    nc.vector.reciprocal(
                        out=d_all[64:65, c0:c0 + 2 * s],
                        in_=o65all[64:65, c0:c0 + 2 * s])

            def emit_norm(g, h, ps_rb):
                # broadcast 1/d across 64 partitions (PE) then multiply
                dc, g0 = h // 2, offs[g]
                s = slot_sizes[g]
                c0 = icol(g, h)
                rb_ps = ps_rb.tile([64, MAX_FREE], F32, tag="rb")
                nc.tensor.matmul(
                    rb_ps[:, :s],
                    ones_t[64:65, :],
                    d_all[64:65, c0:c0 + s],
                    start=True, stop=True)
                with nc.allow_low_precision(reason="bf16 attention output"):
                    nc.vector.tensor_mul(
                        ots[dc][0:64, h % 2, g0:g0 + s],
                        o65all[0:64, c0:c0 + s], rb_ps[:, :s])

            def emit_norm_pair(g, j, ps_rb):
                # both parities in one DVE multiply: rb holds 1/d for the
                # even|odd halves in two banks; the 3D output view writes
                # the even half and the odd staging of the same ots tile
                dc, g0 = j, offs[g]
                s = slot_sizes[g]
                c0 = icol(g, 2 * j)
                pitch = 256 if s <= 256 else MAX_FREE
                rb2 = ps_rb.tile([64, 2, pitch], F32, tag="rbp")
                if s == pitch:
                    # contiguous pair: one broadcast matmul covers both
                    nc.tensor.matmul(
                        rb2[:, :, :].rearrange("p a b -> p (a b)"),
                        ones_t[64:65, :],
                        d_all[64:65, c0:c0 + 2 * s],
                        start=True, stop=True)
                else:
                    for par in range(2):
                        nc.tensor.matmul(
                            rb2[:, par, :s],
                            ones_t[64:65, :],
                            d_all[64:65, c0 + par * s:c0 + par * s + s],
                            start=True, stop=True)
                with nc.allow_low_precision(reason="bf16 attention output"):
                    nc.vector.tensor_mul(
                        ots[dc][0:64, :, g0:g0 + s],
                        o65all[0:64, c0:c0 + 2 * s]
                        .rearrange("p (j w) -> p j w", j=2),
                        rb2[:, :, :s])

            def emit_recip_act(g, j):
                # 1/d = exp(-ln d) on ACT: both functions live in the
                # natural_log_exp_and_others table set (no switch)
                s = slot_sizes[g]
                c0 = icol(g, 2 * j)
                lnd = vp.tile([65, 2 * MAX_FREE], F32, tag="lnd")
                with nc.allow_low_precision(reason="recip via ln/exp"):
                    nc.scalar.activation(
                        out=lnd[64:65, :2 * s],
                        in_=o65all[64:65, c0:c0 + 2 * s],
                        func=mybir.ActivationFunctionType.Ln)
                    nc.scalar.activation(
                        out=d_all[64:65, c0:c0 + 2 * s].bitcast(F32),
                        in_=lnd[64:65, :2 * s],
                        func=mybir.ActivationFunctionType.Exp,
                        scale=-1.0)

            def emit_recip_direct(g, h, o_ps):
                # reciprocal straight from the AV psum's denominator row
                s = slot_sizes[g]
                c0 = icol(g, h)
                with nc.allow_low_precision(
                        reason="f32r rounding for PE broadcast"):
                    nc.vector.reciprocal(
                        out=d_all[64:65, c0:c0 + s],
                        in_=o_ps[64:65, :s])

            def emit_norm_direct(g, h, o_ps, ps_rb):
                # normalize straight from psum (no o65all staging copy)
                dc, g0 = h // 2, offs[g]
                s = slot_sizes[g]
                c0 = icol(g, h)
                rb_ps = ps_rb.tile([64, MAX_FREE], F32, tag="rb")
                nc.tensor.matmul(
                    rb_ps[:, :s],
                    ones_t[64:65, :],
                    d_all[64:65, c0:c0 + s],
                    start=True, stop=True)
                with nc.allow_low_precision(reason="bf16 attention output"):
                    nc.vector.tensor_mul(
                        ots[dc][0:64, h % 2, g0:g0 + s],
                        o_ps[0:64, :s], rb_ps[:, :s])

            def emit_shift(g, dc):
                # odd-head staging -> partition base 64 (DMA only); on the
                # Pool/SWDGE queue so y DMAs never queue ahead of it
                g0, gw = offs[g], slot_sizes[g]
                nc.gpsimd.dma_start(out=ots[dc][64:128, 0, g0:g0 + gw],
                                    in_=ots[dc][0:64, 1, g0:g0 + gw])

            def emit_yproj_mm(g, mi, ps_y, split_from=KC):
                m0, pm = mchunks[g][mi]
                a0 = offs[g] + m0
                y_ps = ps_y.tile([128, MAX_FREE], F32, tag="y")
                for dc in range(split_from):
                    nc.tensor.matmul(
                        y_ps[:pm, :],
                        ots[dc][:, 0, a0:a0 + pm],
                        wos[dc][:],
                        start=(dc == 0), stop=(dc == KC - 1))
                for dc in range(split_from, KC):
                    # read the odd-head staging directly (partitions 0-63)
                    # instead of waiting for that pair's shift DMA
                    nc.tensor.matmul(
                        y_ps[:pm, :],
                        ots[dc][0:64, 0, a0:a0 + pm],
                        wos[dc][0:64, :],
                        start=False, stop=False)
                    nc.tensor.matmul(
                        y_ps[:pm, :],
                        ots[dc][0:64, 1, a0:a0 + pm],
                        wo_odd_t[:, dc - 2, :],
                        start=False, stop=(dc == KC - 1))
                return y_ps

            def emit_yevict(g, mi, y_ps, evict_dve=False, split=False):
                _, pm = mchunks[g][mi]
                with nc.allow_low_precision(reason="bf16 output staging"):
                    if split:
                        # halves on both engines in parallel (tail latency)
                        nc.scalar.copy(out=y_sb[g][:pm, mi, :256],
                                       in_=y_ps[:pm, :256])
                        nc.vector.tensor_copy(out=y_sb[g][:pm, mi, 256:],
                                              in_=y_ps[:pm, 256:])
                    elif evict_dve:
                        nc.vector.tensor_copy(out=y_sb[g][:pm, mi, :],
                                              in_=y_ps[:pm, :])
                    else:
                        nc.scalar.copy(out=y_sb[g][:pm, mi, :],
                                       in_=y_ps[:pm, :])

            def emit_yproj(g, mi, ps_y, split_from=KC, evict_dve=False,
                           evict_split=False):
                y_ps = emit_yproj_mm(g, mi, ps_y, split_from)
                emit_yevict(g, mi, y_ps, evict_dve, split=evict_split)

            def emit_yproj_tail(g, mi, ps_y, split_from):
                # last chunk: column-halved matmuls/evicts/DMAs so the two
                # evictions run on ACT||DVE and the first half-DMA issues
                # while the second half is still projecting
                m0, pm = mchunks[g][mi]
                a0 = offs[g] + m0
                y_ps2 = ps_y.tile([128, 2, 256], F32, tag="y")
                for half in range(2):
                    cs = slice(half * 256, half * 256 + 256)
                    y_ps = y_ps2[:, half]
                    for dc in range(split_from):
                        nc.tensor.matmul(
                            y_ps[:pm, :],
                            ots[dc][:, 0, a0:a0 + pm],
                            wos[dc][:, cs],
                            start=(dc == 0), stop=(dc == KC - 1))
                    for dc in range(split_from, KC):
                        nc.tensor.matmul(
                            y_ps[:pm, :],
                            ots[dc][0:64, 0, a0:a0 + pm],
                            wos[dc][0:64, cs],
                            start=False, stop=False)
                        nc.tensor.matmul(
                            y_ps[:pm, :],
                            ots[dc][0:64, 1, a0:a0 + pm],
                            wo_odd_t[:, dc - 2, cs],
                            start=False, stop=(dc == KC - 1))
                    with nc.allow_low_precision(reason="bf16 staging"):
                        if half == 0:
                            nc.scalar.copy(out=y_sb[g][:pm, mi, cs],
                                           in_=y_ps[:pm, :])
                        else:
                            nc.vector.tensor_copy(out=y_sb[g][:pm, mi, cs],
                                                  in_=y_ps[:pm, :])
                    dst = y[aoff[g] + mi * 128:aoff[g] + (mi + 1) * 128, cs]
                    nc.sync.dma_start(out=dst, in_=y_sb[g][:, mi, cs])

            def emit_ydma(g, mi=None):
                if mi is None:
                    dst = y[aoff[g]:aoff[g] + mg[g] * 128, :]
                    nc.sync.dma_start(
                        out=dst.rearrange("(m p) n -> p m n", p=128),
                        in_=y_sb[g][:])
                else:
                    dst = y[aoff[g] + mi * 128:aoff[g] + (mi + 1) * 128, :]
                    nc.sync.dma_start(out=dst, in_=y_sb[g][:, mi])

            def run_slot(g, ps_s, ps_o, ps_rb, inject, start_av=2,
                         copy_act_odd=False, split_from=KC,
                         direct_norm=False, recip_act=False,
                         copy_act_all=False, merge_mul=False,
                         merge_exp=False, recip_psum=False):
                """Emit slot g's pipelined item stream.

                inject: dict item_index -> list of thunks to emit there
                (e.g. QK projection groups, V projections, prev slot's
                yproj). Engines execute their queues in order, so this
                emission order IS the per-engine execution order.
                """
                pend = []          # (h, e_t) awaiting AV
                done_pairs = []    # pairs awaiting norm emission
                o_tiles = {}       # h -> AV psum tile (direct_norm)
                norm_pend = []     # pairs with recip emitted, mul pending
                av_done = 0

                def pop_av():
                    nonlocal av_done
                    h, e_t = pend.pop(0)
                    o_tiles[h] = emit_av(
                        g, h, e_t, ps_o, no_copy=direct_norm,
                        copy_act=copy_act_all
                        or (copy_act_odd and h % 2 == 1))
                    av_done += 1
                    if av_done % 2 == 0:
                        done_pairs.append(av_done // 2 - 1)

                def pop_norm():
                    j = done_pairs.pop(0)
                    if direct_norm:
                        emit_recip_direct(g, 2 * j, o_tiles[2 * j])
                        emit_recip_direct(g, 2 * j + 1, o_tiles[2 * j + 1])
                        emit_norm_direct(g, 2 * j, o_tiles[2 * j], ps_rb)
                        emit_norm_direct(g, 2 * j + 1, o_tiles[2 * j + 1],
                                         ps_rb)
                    else:
                        if recip_act:
                            emit_recip_act(g, j)
                        elif recip_psum:
                            # straight from the AV psum denominator rows:
                            # runs concurrently with the o65 staging copy
                            emit_recip_direct(g, 2 * j, o_tiles[2 * j])
                            emit_recip_direct(g, 2 * j + 1,
                                              o_tiles[2 * j + 1])
                        else:
                            emit_recip(g, j)
                        if merge_mul:
                            emit_norm_pair(g, j, ps_rb)
                        else:
                            emit_norm(g, 2 * j, ps_rb)
                            emit_norm(g, 2 * j + 1, ps_rb)
                    if j < split_from:
                        emit_shift(g, j)

                s = slot_sizes[g]
                pair_ctx = [None]

                def scores_item(h):
                    if not merge_exp:
                        return emit_scores(g, h, ps_s)
                    # pair-merged exp: both heads' scores in one contiguous
                    # 2-bank tile, ONE activation covers the pair
                    if h % 2 == 0:
                        sps = ps_s.tile([128, 2, mg[g], 256], F32,
                                        tag=f"sps{g}")
                        et = ep.tile([128, 2, mg[g], s], F32R, tag=f"e{g}")
                        pair_ctx[0] = (sps, et)
                    sps, et = pair_ctx[0]
                    dc, r0, g0 = h // 2, (h % 2) * 64, offs[g]
                    for mi, (m0, pm) in enumerate(mchunks[g]):
                        nc.tensor.matmul(
                            sps[:, h % 2, mi, :s],
                            kts[dc][r0:r0 + 64, g0 + m0:g0 + m0 + 128],
                            qts[dc][r0:r0 + 64, g0:g0 + s],
                            start=True, stop=True)
                    if h % 2 == 1:
                        nc.scalar.activation(
                            out=et[:, :, :, :]
                            .rearrange("p a b c -> p (a b c)"),
                            in_=sps[:, :, :, :]
                            .rearrange("p a b c -> p (a b c)"),
                            func=mybir.ActivationFunctionType.Exp,
                            scale=0.125)
                    return et[:, h % 2]

                for h in range(NUM_HEADS):
                    for th in inject.get(h, ()):
                        th()
                    pend.append((h, scores_item(h)))
                    if h >= start_av:
                        pop_av()
                    if h >= start_av + 2 and done_pairs:
                        pop_norm()
                for th in inject.get(NUM_HEADS, ()):
                    th()
                while pend:
                    pop_av()
                    if done_pairs:
                        pop_norm()
                while done_pairs:
                    pop_norm()
                while norm_pend:
                    jj = norm_pend.pop(0)
                    emit_norm_pair(g, jj, ps_rb)
                    if jj < split_from:
                        emit_shift(g, jj)

            # ---- emission ---------------------------------------------
            # PSUM banks: qk(2) + s0(mg0) + o(2) + rb(1) <= 8 during the
            # front phase; s1(mg1) + y(2) + o(2) + rb(1) <= 8 in the back.
            assert mg[0] + 5 <= 8, f"slot0 too large: {mg[0]} score banks"
            if G == 2:
                assert mg[1] + 5 <= 8, f"slot1 too large: {mg[1]} banks"
            with tc.tile_pool(name="ps_o", bufs=2, space="PSUM") as ps_o:
                with (
                    tc.tile_pool(name="ps_qk", bufs=2, space="PSUM") as ps_qk,
                    tc.tile_pool(name="ps_s0", bufs=1, space="PSUM") as ps_s0,
                    tc.tile_pool(name="ps_rb", bufs=1, space="PSUM") as ps_rb,
                ):
                    # slot0 items interleaved with QK projection groups:
                    # heads 2dc..2dc+1 follow the dc they depend on.
                    inj = {}
                    for dc in range(KC):
                        th = []
                        for i, (n0, nw) in enumerate(n_tiles_all):
                            th.append(lambda dc=dc, n0=n0, nw=nw:
                                      emit_qk_group(qts, wq_all, bq_t, dc, 0,
                                                    n0, nw, ps_qk, True))
                            th.append(lambda dc=dc, n0=n0, nw=nw:
                                      emit_qk_group(kts, wk_all, bk_t, dc, 0,
                                                    n0, nw, ps_qk, False))
                        inj[2 * dc] = th
                    # V projections at h=4..7: by then the PE pstate ramp
                    # is warm (these are 512-wide moving operands)
                    for i in range(mg[0]):
                        inj.setdefault(2, []).append(
                            lambda mi=i: emit_v(0, mi, ps_o))
                    if G == 2:
                        for i in range(mg[1]):
                            inj.setdefault(4 + min(i, 1), []).append(
                                lambda mi=i: emit_v(1, mi, ps_o))
                    run_slot(0, ps_s0, ps_o, ps_rb, inj, start_av=4,
                             copy_act_odd=True,
                             split_from=(2 if G == 1 else KC))

                if G == 1:
                    with tc.tile_pool(name="ps_y", bufs=2,
                                      space="PSUM") as ps_y:
                        for mi in range(mg[0]):
                            emit_yproj(0, mi, ps_y, split_from=2,
                                       evict_dve=(mi % 2 == 1))
                            emit_ydma(0, mi)
                else:
                    with (
                        tc.tile_pool(name="ps_s1", bufs=1,
                                     space="PSUM") as ps_s1,
                        tc.tile_pool(name="ps_y", bufs=2,
                                     space="PSUM") as ps_y,
                        tc.tile_pool(name="ps_rb2", bufs=2,
                                     space="PSUM") as ps_rb2,
                    ):
                        def yp0(mi):
                            emit_yproj(0, mi, ps_y)
                            emit_ydma(0, mi)

                        inj = {5 + i: [lambda mi=mi: yp0(mi)]
                               for i, mi in enumerate(range(mg[0]))}
                        run_slot(1, ps_s1, ps_o, ps_rb2, inj,
                                 copy_act_all=True, split_from=2,
                                 merge_mul=True,
                                 merge_exp=(slot_sizes[1] <= 256))
                        for mi in range(mg[1]):
                            emit_yproj(1, mi, ps_y, split_from=2,
                                       evict_dve=(mi % 2 == 0))
                            emit_ydma(1, mi)

    nc.compile()
    return nc


def _plan(batch):
    """Assign whole graphs (contiguous segments) to cores/slots.

    Returns (slot_sizes, assign) where assign[core][slot] = (start, size)
    of the graph segment in the global node order (size 0 = empty slot).
    """
    batch = np.asarray(batch)
    vals, starts, counts = np.unique(batch, return_index=True,
                                     return_counts=True)
    segs = sorted(zip(starts.tolist(), counts.tolist()),
                  key=lambda t: -t[1])
    n_slots = _ceil_div(len(segs), N_CORES)
    while len(segs) < n_slots * N_CORES:
        segs.append((0, 0))
    assign = [[None] * n_slots for _ in range(N_CORES)]
    slot_sizes = []
    for j in range(n_slots):
        block = segs[j * N_CORES:(j + 1) * N_CORES]
        order = range(N_CORES) if j % 2 == 0 else range(N_CORES - 1, -1, -1)
        for c, k in zip(order, range(N_CORES)):
            assign[c][j] = block[k]
        m = max(sz for (_, sz) in block)
        m = _ceil_div(m, 4) * 4
        # f32r matmuls drop to 4 cycles/row below a 256-wide moving free
        # dim — pad mid-sized slots up to 256 to stay on the fast path
        if m >= 64:
            m = max(m, 256)
        slot_sizes.append(m)
    keep = [j for j, s in enumerate(slot_sizes) if s > 0]
    slot_sizes = [slot_sizes[j] for j in keep]
    assign = [[assign[c][j] for j in keep] for c in range(N_CORES)]
    return tuple(slot_sizes), assign


def kernel(x, batch, Wq, bq, Wk, bk, Wv, bv, Wo, bo):
    out, _ = _execute(dict(x=x, batch=batch, Wq=Wq, bq=bq, Wk=Wk, bk=bk,
                           Wv=Wv, bv=bv, Wo=Wo, bo=bo))
    return out


def _aligned_offsets(slot_sizes):
    aoff = [0]
    for s in slot_sizes:
        aoff.append(aoff[-1] + _ceil_div(s, 128) * 128)
    return aoff


def _prepare(inputs):
    x = np.ascontiguousarray(np.asarray(inputs["x"], dtype=np.float32))
    Wq = np.asarray(inputs["Wq"], dtype=np.float32)
    Wk = np.asarray(inputs["Wk"], dtype=np.float32)
    Wv = np.asarray(inputs["Wv"], dtype=np.float32)
    Wo = np.asarray(inputs["Wo"], dtype=np.float32)
    bq = np.asarray(inputs["bq"], dtype=np.float32)
    bk = np.asarray(inputs["bk"], dtype=np.float32)
    bv = np.asarray(inputs["bv"], dtype=np.float32)
    bo = np.asarray(inputs["bo"], dtype=np.float32)

    slot_sizes, assign = _plan(inputs["batch"])
    offs = np.concatenate([[0], np.cumsum(slot_sizes)]).astype(int)
    nc_tot = int(offs[-1])
    aoff = _aligned_offsets(slot_sizes)

    bf = ml_dtypes.bfloat16
    def _pack(W):
        t = np.ascontiguousarray(W.T).astype(bf)
        return np.ascontiguousarray(
            t.reshape(KC, 128, KC, 128).transpose(1, 2, 0, 3)
            .reshape(128, KC * HIDDEN))
    wqT = _pack(Wq)
    wkT = _pack(Wk)
    wvT = np.ascontiguousarray(Wv.T).astype(bf)
    woT = np.ascontiguousarray(Wo.T).astype(bf)
    # V-bias and out-bias fold: softmax rows sum to 1, so attn@(V+bv) =
    # attn@V + bv, and (O+bv)@Wo.T + bo = O@Wo.T + (Wo@bv + bo).
    b_out = (Wo @ bv + bo).astype(np.float32)
    bqk = np.concatenate([bq.reshape(KC, 128), bk.reshape(KC, 128)], axis=0)
    bqk = np.ascontiguousarray(bqk)

    in_maps = []
    for c in range(N_CORES):
        xT = np.zeros((HIDDEN, nc_tot), dtype=np.float32)
        von = np.zeros((aoff[-1],), dtype=np.float32)
        for j, (st, sz) in enumerate(assign[c]):
            if sz:
                xT[:, offs[j]:offs[j] + sz] = x[st:st + sz].T
                von[aoff[j]:aoff[j] + sz] = 1.0
        in_maps.append({
            "xT": xT.astype(bf), "wq": wqT, "wk": wkT, "wv": wvT, "wo": woT,
            "wo_odd": np.ascontiguousarray(
                np.concatenate([woT[320:384, :], woT[448:512, :]], axis=0)),
            "bqk": bqk, "vones": von,
        })
    return slot_sizes, assign, offs, in_maps, b_out


def _gather(results, assign, offs, n_nodes, b_out, slot_sizes):
    aoff = _aligned_offsets(slot_sizes)
    out = np.empty((n_nodes, HIDDEN), dtype=np.float32)
    for c in range(N_CORES):
        yc = np.asarray(results[c]["y"])
        if yc.dtype != np.float32:
            yc = yc.view(ml_dtypes.bfloat16).astype(np.float32) \
                if yc.dtype.itemsize == 2 else yc.astype(np.float32)
        else:
            yc = yc
        for j, (st, sz) in enumerate(assign[c]):
            if sz:
                out[st:st + sz] = yc[aoff[j]:aoff[j] + sz]
    out += b_out[None, :]
    return out


def _execute(inputs, trace=False, **run_kwargs):
    slot_sizes, assign, offs, in_maps, b_out = _prepare(inputs)
    if slot_sizes not in _CACHE:
        _CACHE[slot_sizes] = _build_program(list(slot_sizes))
    nc = _CACHE[slot_sizes]
    res = run_bass_kernel_spmd(nc, in_maps, list(range(N_CORES)),
                               trace=trace, **run_kwargs)
    out = _gather(res.results, assign, offs,
                  np.asarray(inputs["x"]).shape[0], b_out, slot_sizes)
    return out, res
